# revision 23
# baseline (speedup 1.0000x reference)
"""MultiHeadMemory kernel for 8 Trainium2 NeuronCores.

Math (reference):
    mem_key = softmax(mems @ Wk + bk)           [H, M, KD]
    mem_val = mems @ Wv + bv                    [H, M, VD]
    att     = k @ mem_key[h].T                  [H, N, M]
    att_w   = softmax(att, axis=-1)
    out_h   = att_w @ mem_val[h]                [H, N, VD]
    out     = concat_h(out_h) @ Wf + bf         [N, VD]

Restructuring used here:
    out = sum_h att_w_h @ (mem_val_h @ Wf_h) + bf
        = sum_h att_w_h @ MVP'_h           with MVP'_h = mem_val_h @ Wf_h + bf/8
    (each head's att_w rows sum to 1, so the bf/8 terms add back bf exactly)

Device kernel per 512-row superblock (4 sub-blocks of 128 rows):
    att   = k_blk @ MKT        (PE, F=512; MKT = stacked mem_key.T, static)
    e     = exp(att)           (ACT, PSUM->SBUF, fp16; |att| < ~1 so no max)
    s     = per-head sums      (DVE pairwise fold tree, fp16 2x mode)
    r     = 1/s                (DVE)
    att_w = e * r[bcast]       (28/32 groups DVE + 4/32 GPSIMD, broadcast AP)
    awT   = att_w.T            (8x PE transpose of fp16-PAIRS viewed as f32;
                                j-pair permutation folded into MVP row order)
    awT_sb (copy split: DVE half + ACT half)
    outT  += MVP_c.T @ awT_c   (PE, static fp16 weights, F=512 strided fp16
                                view of the packed awT, f32 PSUM)
    store (evac split ACT/DVE), DMA outT [VD, n] slices.

The kernel writes the TRANSPOSED output outT [VD, nshard]; the host
re-transposes. This lets mm2 run with static weights and 512-wide moving
operands (4 matmuls instead of 16 per superblock).

Perf notes (probe-measured on axon trn2): the original kernel was
bottlenecked by the GPSIMD normalize (~3.2us/superblock) and the DVE
PSUM->SBUF copy; splitting both across engines, packing transposes in
f32 pairs (16 -> 8), and deepening SBUF pools (bufs=5) took the
measured repeat-slope device time from ~270-296us to ~92us.

Sharding: data-parallel on N across 8 cores; weights replicated.
Host precomputes MKT/MVP from the <1MB weight tensors and pre-transposes k
so the contraction dim of mm1 lands on SBUF partitions.
"""

import os
import sys

import numpy as np

sys.path.insert(0, "/opt/trn_rl_repo")

import concourse.bass as bass
import concourse.mybir as mybir
import concourse.tile as tile
from concourse import bacc
from concourse.bass_utils import run_bass_kernel_spmd

H, M, D, KD, VD = 8, 64, 128, 128, 128
N_FULL = 262144
NCORES = 8
NSHARD = N_FULL // NCORES  # 32768
SUPER = 512                # rows per superblock (1 DMA in / 1 DMA out)
NSUB = SUPER // 128        # 4 sub-blocks of 128 rows
JD = H * M                 # 512 (head, mem-slot) pairs
NCHUNK = JD // 128         # 4 contraction chunks for mm2
NG = NSUB * H              # 32 softmax groups per superblock

F32 = mybir.dt.float32
FP16 = mybir.dt.float16

# Input dtype is host-side (affects in_maps); keep it env-level.
KT_DT = FP16 if os.environ.get("KMHM_KT_DT", "fp16") == "fp16" else F32

# Build-time knob defaults; build_kernel(**over) can override per build.
DEFAULTS = {
    "NORM_ENGINE": "gpsimd",  # engine for broadcast normalize: gpsimd|vector
    "NORM_SPLIT": 28,         # >0: first N of 32 groups on DVE, rest on gpsimd
    "BUFS": 5,
    "OUT_DT": "f32",          # f32|fp16
    "TREE": "tree",           # tree|reduce  (pairwise fold vs tensor_reduce)
    "TREE_SPLIT": 0,
    "EVAC": "split",          # split|scalar|vector
    "AWT_SPLIT": 1,
    "AWT_DVE_N": 512,         # f32 elems of awt copy on DVE (rest scalar), if AWT_SPLIT
    "EVAC_SC_N": 256,         # f32 elems of evac on scalar (rest vector), if EVAC=split
    "TRANS_PACK": 1,          # 1 = fp16-pair-as-f32 packed transposes (8/superblock)
    "TRANS_DUP_N": 0,         # extra redundant transposes per superblock (PE keep-alive)
    "KTBUFS": 6,
    "ATTPS_BUFS": 2,
    "DMA_OUT": "sync",        # engine queue for output store: sync|scalar|vector|gpsimd
    "SKIP": "",               # timing probes: stages to omit (wrong results)
    "DUP": "",                # timing probes: stages to run twice
}


def _cfg(over):
    cfg = {}
    for k, dflt in DEFAULTS.items():
        v = over.get(k, os.environ.get(f"KMHM_{k}", dflt))
        cfg[k] = type(dflt)(v) if not isinstance(dflt, str) else str(v)
    return cfg


def _host_precompute(mems, Wk, bk, Wv, bv, Wf, bf):
    """MKT [KD, H*M] and MVP [H*M, VD] in float64, returned as float32."""
    mems = mems.astype(np.float64)
    Wk = Wk.astype(np.float64)
    bk = bk.astype(np.float64)
    Wv = Wv.astype(np.float64)
    bv = bv.astype(np.float64)
    Wf = Wf.astype(np.float64)
    bf = bf.astype(np.float64)

    logits = np.einsum("hmd,hdk->hmk", mems, Wk) + bk[:, None, :]
    logits -= logits.max(axis=-1, keepdims=True)
    e = np.exp(logits)
    mem_key = e / e.sum(axis=-1, keepdims=True)          # [H, M, KD]
    mem_val = np.einsum("hmd,hdv->hmv", mems, Wv) + bv[:, None, :]  # [H, M, VD]

    Wf_h = Wf.reshape(H, VD, VD)                          # [H, VD, VD]
    mvp = np.einsum("hmv,hvu->hmu", mem_val, Wf_h)        # [H, M, VD]
    mvp = mvp + bf[None, None, :] / H

    mkt = mem_key.reshape(JD, KD).T                       # [KD, H*M]
    mvp = mvp.reshape(JD, VD)                             # [H*M, VD]
    return (
        np.ascontiguousarray(mkt, dtype=np.float32),
        np.ascontiguousarray(mvp, dtype=np.float32),
    )


def build_kernel(nshard=NSHARD, repeat=1, **over):
    cfg = _cfg(over)
    SKIP = set(filter(None, cfg["SKIP"].split(",")))
    DUP = set(filter(None, cfg["DUP"].split(",")))
    NBUFS = cfg["BUFS"]
    OUT_DT = FP16 if cfg["OUT_DT"] == "fp16" else F32
    PACK = cfg["TRANS_PACK"] == 1

    nc = bacc.Bacc()
    kt = nc.declare_dram_parameter("kt", [KD, nshard], KT_DT, isOutput=False)
    mkt = nc.declare_dram_parameter("mkt", [KD, JD], KT_DT, isOutput=False)
    mvp = nc.declare_dram_parameter("mvp", [JD, VD], FP16, isOutput=False)
    ident = nc.declare_dram_parameter("ident", [128, 128], FP16, isOutput=False)
    outT = nc.declare_dram_parameter("outT", [VD, nshard], OUT_DT, isOutput=True)

    nsuper = nshard // SUPER
    norm_eng = getattr(nc, {"gpsimd": "gpsimd", "vector": "vector"}[cfg["NORM_ENGINE"]])

    with tile.TileContext(nc) as tc:
        with (
            tc.tile_pool(name="const", bufs=1) as const,
            tc.tile_pool(name="ktp", bufs=cfg["KTBUFS"]) as kt_pool,
            tc.tile_pool(name="expp", bufs=NBUFS) as exp_pool,
            tc.tile_pool(name="treep", bufs=NBUFS) as tree_pool,
            tc.tile_pool(name="statp", bufs=4) as stat_pool,
            tc.tile_pool(name="attwp", bufs=NBUFS) as attw_pool,
            tc.tile_pool(name="awtp", bufs=NBUFS) as awt_pool,
            tc.tile_pool(name="osbp", bufs=cfg["KTBUFS"]) as osb_pool,
            tc.tile_pool(name="attps", bufs=cfg["ATTPS_BUFS"], space="PSUM") as att_ps_pool,
            tc.tile_pool(name="awtps", bufs=1, space="PSUM") as awt_ps_pool,
            tc.tile_pool(name="outps", bufs=2, space="PSUM") as out_ps_pool,
        ):
            mkt_sb = const.tile([KD, JD], KT_DT)
            nc.gpsimd.dma_start(out=mkt_sb, in_=mkt[:])
            # MVP chunks as mm2 stationary: [j_local(128), c(4), v(128)]
            # PACK: chunk c' = (a, p), row t <- global j = 256*a + 2*t + p,
            # matching the fp16-pair-as-f32 packed transpose layout.
            mvp_sb = const.tile([128, NCHUNK, VD], FP16)
            if PACK:
                nc.gpsimd.dma_start(
                    out=mvp_sb.rearrange("t (a p) v -> t a p v", a=2),
                    in_=mvp[:].rearrange("(a t p) v -> t a p v", a=2, p=2),
                )
            else:
                nc.gpsimd.dma_start(
                    out=mvp_sb, in_=mvp[:].rearrange("(c j) v -> j c v", j=128)
                )
            id_sb = const.tile([128, 128], FP16)
            nc.gpsimd.dma_start(out=id_sb, in_=ident[:])
            if PACK:
                id32_sb = const.tile([128, 128], F32)
                nc.vector.tensor_copy(id32_sb, id_sb)

            for rep in range(repeat):
              for sb in range(nsuper):
                kt_t = kt_pool.tile([KD, SUPER], KT_DT)
                nc.sync.dma_start(
                    out=kt_t, in_=kt[:, sb * SUPER:(sb + 1) * SUPER]
                )
                # Superblock-wide tiles
                exp_t = exp_pool.tile([128, NSUB * JD], FP16)   # [p,(g,m)]
                tree_t = tree_pool.tile([128, 2048], FP16)
                sums_t = stat_pool.tile([128, NG], F32, tag="sums")
                recip_t = stat_pool.tile([128, NG], F32, tag="recip")
                attw_t = attw_pool.tile([128, NSUB * JD], FP16)
                # awt layout: [p, (c, s, n)] so mm2 chunk c streams F=512
                if PACK:
                    awt_ps = awt_ps_pool.tile([128, NSUB * JD // 2], F32)
                    awt_t = awt_pool.tile([128, NSUB * JD // 2], F32)
                else:
                    awt_ps = awt_ps_pool.tile([128, NSUB * JD], FP16)
                    awt_t = awt_pool.tile([128, NSUB * JD], FP16)
                out_ps = out_ps_pool.tile([128, SUPER], F32)
                out_sb = osb_pool.tile([128, SUPER], OUT_DT)

                for half in range(2 * (2 if "mm1" in DUP else 1)):
                    half = half % 2
                    att_ps = att_ps_pool.tile([128, 2 * JD], F32, tag="attps")
                    for i in range(2):
                        s = half * 2 + i
                        nc.tensor.matmul(
                            att_ps[:, i * JD:(i + 1) * JD],
                            lhsT=kt_t[:, s * 128:(s + 1) * 128],
                            rhs=mkt_sb,
                            start=True, stop=True,
                        )
                    # exp (PSUM -> SBUF, f32 -> fp16)
                    for _e in range(2 if "exp" in DUP else 1):
                        nc.scalar.activation(
                            exp_t[:, half * 2 * JD:(half + 1) * 2 * JD], att_ps,
                            mybir.ActivationFunctionType.Exp,
                        )

                # --- per-group sums ---
                ev = exp_t.rearrange("p (g m) -> p g m", m=M)
                if "tree" in SKIP:
                    nc.vector.memset(sums_t, 1.0)
                    nc.vector.reciprocal(recip_t, sums_t)
                elif cfg["TREE"] == "reduce":
                    for _l1 in range(2 if "tree" in DUP else 1):
                        nc.vector.reduce_sum(
                            sums_t, ev, axis=mybir.AxisListType.X
                        )
                    nc.vector.reciprocal(recip_t, sums_t)
                else:
                    t1 = tree_t[:, 0:1024].rearrange("p (g m) -> p g m", m=32)
                    for _l1 in range(2 if "tree" in DUP else 1):
                        nc.vector.tensor_add(t1, ev[:, :, 0:32], ev[:, :, 32:64])
                    tree_eng = nc.gpsimd if cfg["TREE_SPLIT"] else nc.vector
                    t2 = tree_t[:, 1024:1536].rearrange("p (g m) -> p g m", m=16)
                    tree_eng.tensor_add(t2, t1[:, :, 0:16], t1[:, :, 16:32])
                    t3 = tree_t[:, 1536:1792].rearrange("p (g m) -> p g m", m=8)
                    tree_eng.tensor_add(t3, t2[:, :, 0:8], t2[:, :, 8:16])
                    t4 = tree_t[:, 1792:1920].rearrange("p (g m) -> p g m", m=4)
                    tree_eng.tensor_add(t4, t3[:, :, 0:4], t3[:, :, 4:8])
                    t5 = tree_t[:, 1920:1984].rearrange("p (g m) -> p g m", m=2)
                    tree_eng.tensor_add(t5, t4[:, :, 0:2], t4[:, :, 2:4])
                    tree_eng.tensor_add(
                        sums_t.rearrange("p (g m) -> p g m", m=1),
                        t5[:, :, 0:1], t5[:, :, 1:2],
                    )
                    nc.vector.reciprocal(recip_t, sums_t)

                # --- normalize: att_w = e * r (broadcast r over m) ---
                rb = bass.AP(
                    tensor=recip_t.tensor,
                    offset=recip_t.offset,
                    ap=[recip_t.ap[0], [1, NG], [0, M]],
                )
                if "norm" not in SKIP:
                    aw = attw_t.rearrange("p (g m) -> p g m", m=M)
                    ns = cfg["NORM_SPLIT"]
                    for _n in range(2 if "norm" in DUP else 1):
                        if ns > 0:
                            rb_lo = recip_t[:, 0:ns].unsqueeze(-1).broadcast_to(
                                (128, ns, M))
                            rb_hi = recip_t[:, ns:].unsqueeze(-1).broadcast_to(
                                (128, NG - ns, M))
                            nc.vector.tensor_mul(
                                aw[:, 0:ns, :], ev[:, 0:ns, :], rb_lo)
                            nc.gpsimd.tensor_mul(
                                aw[:, ns:, :], ev[:, ns:, :], rb_hi)
                        else:
                            norm_eng.tensor_mul(aw, ev, rb)

                # --- transpose att_w chunks on PE ---
                if PACK:
                    # fp16 pairs as f32: 8 transposes of [128,128] f32.
                    # attw32[:, s*256 + a*128 + t] = pair (j=256a+2t, +1) of
                    # sub-block s; transposed block (s, a) lands at
                    # awt32[:, a*512 + s*128 : ...].
                    attw32 = attw_t[:].bitcast(F32)
                    ndup = cfg["TRANS_DUP_N"]
                    ti = 0
                    for _t in range(2 if "trans" in DUP else 1):
                      for s in (range(NSUB) if "trans" not in SKIP else []):
                        for a in range(2):
                            for _rep2 in range(
                                    2 if (ndup and ti % (8 // ndup) == 0) else 1):
                                nc.tensor.transpose(
                                    awt_ps[:, a * 512 + s * 128:
                                           a * 512 + (s + 1) * 128],
                                    attw32[:, s * 256 + a * 128:
                                           s * 256 + (a + 1) * 128],
                                    id32_sb,
                                )
                            ti += 1
                else:
                    for _t in range(2 if "trans" in DUP else 1):
                      for s in (range(NSUB) if "trans" not in SKIP else []):
                        for c in range(NCHUNK):
                            nc.tensor.transpose(
                                awt_ps[:, c * SUPER + s * 128:
                                       c * SUPER + (s + 1) * 128],
                                attw_t[:, s * JD + c * 128:s * JD + (c + 1) * 128],
                                id_sb,
                            )
                if "copy" not in SKIP:
                    if cfg["AWT_SPLIT"]:
                        hw = min(cfg["AWT_DVE_N"] * (2 - PACK), awt_t.shape[1])
                        for _c in range(2 if "copy" in DUP else 1):
                            nc.vector.tensor_copy(awt_t[:, 0:hw], awt_ps[:, 0:hw])
                            if hw < awt_t.shape[1]:
                                nc.scalar.copy(awt_t[:, hw:], awt_ps[:, hw:])
                    else:
                        for _c in range(2 if "copy" in DUP else 1):
                            nc.vector.tensor_copy(awt_t, awt_ps)

                # --- mm2: outT[v, n] += MVP_c.T @ awT_c  (F=512) ---
                if PACK:
                    # awt16[p, (a, twoP, s, n)]: fp16 elem (a,s,n,p') at
                    # 2*(a*512+s*128+n)+p' -> chunk c'=(a,p') strides s:256,n:2
                    awt16 = awt_t[:].bitcast(FP16).rearrange(
                        "p (a s n two) -> p a two s n", a=2, s=NSUB, two=2
                    )
                for c in range(NCHUNK * (2 if "mm2" in DUP else 1)):
                    c = c % NCHUNK
                    if PACK:
                        a, pp = divmod(c, 2)
                        rhs = awt16[:, a, pp, :, :]
                    else:
                        rhs = awt_t[:, c * SUPER:(c + 1) * SUPER]
                    nc.tensor.matmul(
                        out_ps,
                        lhsT=mvp_sb[:, c, :],
                        rhs=rhs,
                        start=(c == 0), stop=(c == NCHUNK - 1),
                    )
                if cfg["EVAC"] == "dma":
                    nc.sync.dma_start(
                        out=outT[:, sb * SUPER:(sb + 1) * SUPER], in_=out_ps
                    )
                    continue
                if cfg["EVAC"] == "split":
                    sc = cfg["EVAC_SC_N"]
                    for _v in range(2 if "evac" in DUP else 1):
                        if sc > 0:
                            nc.scalar.copy(out_sb[:, 0:sc], out_ps[:, 0:sc])
                        nc.vector.tensor_copy(out_sb[:, sc:], out_ps[:, sc:])
                elif cfg["EVAC"] == "vector":
                    nc.vector.tensor_copy(out_sb, out_ps)
                elif cfg["EVAC"] == "gpsimd":
                    nc.gpsimd.tensor_copy(out_sb, out_ps)
                elif cfg["EVAC"] == "gsplit":
                    nc.scalar.copy(out_sb[:, 0:SUPER // 2],
                                   out_ps[:, 0:SUPER // 2])
                    nc.gpsimd.tensor_copy(out_sb[:, SUPER // 2:],
                                          out_ps[:, SUPER // 2:])
                else:
                    nc.scalar.copy(out_sb, out_ps)
                getattr(nc, cfg["DMA_OUT"]).dma_start(
                    out=outT[:, sb * SUPER:(sb + 1) * SUPER], in_=out_sb
                )

    nc.compile()
    return nc


_CACHED = {}


def _get_kernel(nshard):
    if nshard not in _CACHED:
        _CACHED[nshard] = build_kernel(nshard)
    return _CACHED[nshard]


def make_in_map(kt_full, mkt, mvp, nshard, i):
    np_kt = mybir.dt.np(KT_DT)
    return {
        "kt": np.ascontiguousarray(kt_full[:, i * nshard:(i + 1) * nshard]),
        "mkt": mkt.astype(np_kt),
        "mvp": mvp.astype(np.float16),
        "ident": np.eye(128, dtype=np.float16),
    }


def kernel(k, mems, Wk, bk, Wv, bv, Wf, bf, _collect=None):
    k = np.asarray(k)
    n = k.shape[0]
    nshard = n // NCORES
    mkt, mvp = _host_precompute(
        np.asarray(mems), np.asarray(Wk), np.asarray(bk), np.asarray(Wv),
        np.asarray(bv), np.asarray(Wf), np.asarray(bf),
    )
    np_kt = mybir.dt.np(KT_DT)
    kt_full = np.ascontiguousarray(k.T.astype(np_kt))  # [KD, N]

    nc = _get_kernel(nshard)
    in_maps = []
    for i in range(NCORES):
        in_maps.append(make_in_map(kt_full, mkt, mvp, nshard, i))
    res = run_bass_kernel_spmd(nc, in_maps, core_ids=list(range(NCORES)))
    if _collect is not None:
        _collect.append(res)
    out_t = np.concatenate([r["outT"] for r in res.results], axis=1)  # [VD, N]
    return np.ascontiguousarray(out_t.T.astype(np.float32))

